# revision 16
# baseline (speedup 1.0000x reference)
# Trainium2 Bass kernel for nn_FCM_series_1 (gnn_message_passing).
#
# Math (derived from the reference):
#   aggregate(X, WW)[l,b,j] = tanh(-sum_i X[l,b,i] * WW[i,j])
#   T_A  = aggregate(A, WW)                     (12 lags x B rows)
#   U[t] = aggregate(train_init[:,:,t,1], WW)   (13 unique rows per batch;
#          A_N_OLD[la] = U[la], A_0_NEW[la] = U[la+1])
#   out[b,la,j] = P[la,j]*T_A[la,b,j] + Q[la,j]*U[la+1,b,j] + R[la,j]*U[la,b,j]
# with host-computable coefficients
#   P[la,j] = 2 * lambd[la, j%200] / belta[la] * 3**fract[la]
#   Q[la,j] = 3 * lambd[la, j%200] * l[la, j%200] / belta[la]
#   R[la,j] = Q[la,j] * Gamma(a+1)/(6*Gamma(a-2))
#   belta[la] = sum_{k=0..3} Gamma(a+1)/(Gamma(k+1)*Gamma(a-k+1))
#
# Sharding over 8 cores: batch split x2 (16 each), output node dim j split x4
# (300 each). Per core one matmul chain: lhsT=W-chunk tiles, rhs=X^T tiles,
# PSUM-accumulated over 10 k-tiles of 120, in float32r (single-pass fp32 PE
# mode, 4x faster than fp32 LOW_HIGH). W is negated on the host so psum
# already holds -X@W; tanh on ScalarE; coefficient combine on VectorE with
# 0-stride broadcast APs; per-core [300,192] result re-assembled on the host.
#
# HBM layouts are host-repacked to partition-major so every DMA descriptor is
# one large contiguous run per partition; input DMAs are split between the two
# HWDGE queues (sync for W, scalar for X) to double aggregate DMA throughput.

import math

import numpy as np

LAG = 13
B = 32
N = 1200
H = 1.0 / 3.0

PB = 2          # batch shards
PJ = 4          # j shards
BL = B // PB    # 16 batches per core
JL = N // PJ    # 300 output nodes per core
NL = LAG - 1    # 12
CA = NL * BL    # 192 cols: T_A block, col = la*BL + b
CU = LAG * BL   # 208 cols: U block,  col = CA + t*BL + b
C = CA + CU     # 400 matmul moving cols
KT = 120        # contraction tile
NK = N // KT    # 10
JS = 100        # j subtile (psum partition dim)
NJ = JL // JS   # 3
NCH = 2         # input DMA chunks per tensor (5 k-tiles each)

_cached = None


def _gamma(x):
    return math.gamma(x)


def _build_nc():
    import concourse.bacc as bacc
    import concourse.mybir as mybir
    from concourse.tile import TileContext

    f32 = mybir.dt.float32
    f32r = mybir.dt.float32r
    nc = bacc.Bacc(None, target_bir_lowering=False)

    # partition-major repacked inputs (see kernel() for layouts)
    xt = nc.dram_tensor("xt", [KT, NK * C], f32r, kind="ExternalInput")
    wc = nc.dram_tensor("wc", [KT, NK * JL], f32r, kind="ExternalInput")
    coef = nc.dram_tensor("coef", [JS, 3 * NJ * NL], f32, kind="ExternalInput")
    out = nc.dram_tensor("out", [JL, CA], f32, kind="ExternalOutput")

    with TileContext(nc) as tc:
        with (
            tc.tile_pool(name="sb", bufs=1) as pool,
            tc.tile_pool(name="ps", bufs=1, space="PSUM") as pspool,
        ):
            # Input streaming. W is jt-major on the host so each j-subtile's
            # weights arrive as a unit and its epilogue can start while later
            # subtiles still stream. X (needed by every jt) is spread over
            # both queues; the very last item is only jt2's second W half.
            # sync:   Wjt0 | Wjt1 | Wjt2a(k0-4) | Xk8-9 | Wjt2b(k5-9)
            # scalar: Xk0-1 | Xk2-4 | Xk5-7
            w_tiles = [None] * NJ   # per-jt [KT, NK*JS] views
            x_tiles = [None] * NK   # per-k  [KT, C] views
            KJ = NK * JS            # W cols per jt

            def load(eng, dram, c0, ncols, gi):
                g = pool.tile([KT, ncols], f32r, tag=f"g{gi}", name=f"g{gi}")
                eng.dma_start(out=g[:], in_=dram[:, c0:c0 + ncols])
                return g

            wjt0 = load(nc.sync, wc, 0 * KJ, KJ, 0)
            wjt1 = load(nc.sync, wc, 1 * KJ, KJ, 1)
            wjt2a = load(nc.sync, wc, 2 * KJ, 5 * JS, 2)
            xk89 = load(nc.sync, xt, 8 * C, 2 * C, 3)
            wjt2b = load(nc.sync, wc, 2 * KJ + 5 * JS, 5 * JS, 4)
            xk01 = load(nc.scalar, xt, 0 * C, 2 * C, 5)
            xk24 = load(nc.scalar, xt, 2 * C, 3 * C, 6)
            xk57 = load(nc.scalar, xt, 5 * C, 3 * C, 7)
            coef_all = pool.tile([JS, 3 * NJ * NL], f32, tag="coef")
            nc.gpsimd.dma_start(out=coef_all[:], in_=coef[:, :])

            for k in range(NK):
                if k < 2:
                    x_tiles[k] = xk01[:, k * C:(k + 1) * C]
                elif k < 5:
                    x_tiles[k] = xk24[:, (k - 2) * C:(k - 1) * C]
                elif k < 8:
                    x_tiles[k] = xk57[:, (k - 5) * C:(k - 4) * C]
                else:
                    x_tiles[k] = xk89[:, (k - 8) * C:(k - 7) * C]

            def w_slice(jt, k):
                if jt < 2:
                    return (wjt0 if jt == 0 else wjt1)[:, k * JS:(k + 1) * JS]
                if k < 5:
                    return wjt2a[:, k * JS:(k + 1) * JS]
                return wjt2b[:, (k - 5) * JS:(k - 4) * JS]

            # Warm up the PE (HAM clock gate) with throwaway matmuls while
            # the inputs stream in, so the real matmuls run at 2.4 GHz.
            scratch = pool.tile([KT, C], f32, tag="scr")
            nc.vector.memset(scratch[:], 0)
            psw = pspool.tile([JS, C], f32, tag="psw", name="psw")
            for i in range(8):
                nc.tensor.matmul(psw[:], scratch[:, 0:JS], scratch[:],
                                 start=True, stop=True)

            ps = [pspool.tile([JS, C], f32, tag=f"ps{jt}", name=f"ps{jt}")
                  for jt in range(NJ)]
            for jt in range(NJ):
                for k in range(NK):
                    nc.tensor.matmul(
                        ps[jt][:], w_slice(jt, k), x_tiles[k],
                        start=(k == 0), stop=(k == NK - 1),
                    )

            # Replicate the [JS, 12] coefficient vectors to [JS, 192] during
            # the DMA phase (DVE idle) so the combine ops run on flat APs.
            crep = pool.tile([JS, 3 * NJ * CA], f32, tag="crep")
            for i in range(3):
                for jt in range(NJ):
                    src = coef_all[:, i * NJ * NL + jt * NL:
                                   i * NJ * NL + (jt + 1) * NL]
                    dst = crep[:, (i * NJ + jt) * CA:(i * NJ + jt + 1) * CA]
                    nc.vector.tensor_copy(
                        dst.rearrange("p (l b) -> p l b", b=BL),
                        src.broadcast_to([JS, NL, BL]))

            # Per-jt epilogue, pipelined: tanh on ACT, flat combine on DVE
            # (jt0, jt2) / GpSimd (jt1), per-jt output DMA.
            t_all = pool.tile([JS, NJ * C], f32, tag="t")
            res = pool.tile([JS, NJ * CA], f32, tag="res")
            tmp = pool.tile([JS, NJ * CA], f32, tag="tmp")
            out3 = out.rearrange("(j p) c -> p j c", p=JS)
            for jt in range(NJ):
                # W was negated on the host, so psum = -(X @ W) already.
                nc.scalar.activation(
                    out=t_all[:, jt * C:(jt + 1) * C], in_=ps[jt][:],
                    func=mybir.ActivationFunctionType.Tanh,
                )
                t0 = jt * C
                tA = t_all[:, t0:t0 + CA]
                tU1 = t_all[:, t0 + CA + BL:t0 + CA + CU]
                tU0 = t_all[:, t0 + CA:t0 + CA + CA]
                rs = res[:, jt * CA:(jt + 1) * CA]
                ts = tmp[:, jt * CA:(jt + 1) * CA]
                cof = [crep[:, (i * NJ + jt) * CA:(i * NJ + jt + 1) * CA]
                       for i in range(3)]
                ve = nc.vector if jt != 1 else nc.gpsimd
                ve.tensor_mul(rs, cof[0], tA)
                ve.tensor_mul(ts, cof[1], tU1)
                ve.tensor_add(rs, rs, ts)
                ve.tensor_mul(ts, cof[2], tU0)
                ve.tensor_add(rs, rs, ts)
                oeng = nc.sync if jt != 1 else nc.scalar
                oeng.dma_start(out=out3[:, jt, :], in_=rs)

    return nc


def _get_nc():
    global _cached
    if _cached is None:
        _cached = _build_nc()
        _cached.finalize()   # Bacc: runs reg alloc + codegen passes
    return _cached


def _host_coefs(alpha, fract, lambd, l):
    # All [12,...] fp32; compute in float64, cast at the end.
    a = alpha[:, 0].astype(np.float64)          # [12]
    f = fract[:, 0].astype(np.float64)          # [12]
    lam = lambd[:, 0, :, 0].astype(np.float64)  # [12, 200]
    ll = l[:, 0, :, 0].astype(np.float64)       # [12, 200]

    belta = np.zeros(NL)
    for la in range(NL):
        g_a1 = _gamma(a[la] + 1.0)
        belta[la] = sum(
            g_a1 / (_gamma(kk + 1.0) * _gamma(a[la] - kk + 1.0)) for kk in range(4)
        )
    cN = np.array([_gamma(a[la] + 1.0) / (6.0 * _gamma(a[la] - 2.0))
                   for la in range(NL)])

    # tile lambda/l from 200 -> 1200 (index n % 200)
    lam_t = np.tile(lam, (1, 6))                # [12, 1200]
    ll_t = np.tile(ll, (1, 6))                  # [12, 1200]

    inv_hf = (1.0 / H) ** f                     # 3**fract
    P = 2.0 * lam_t / belta[:, None] * inv_hf[:, None]
    Q = lam_t * ll_t / belta[:, None] / H
    R = Q * cN[:, None]
    return P.astype(np.float32), Q.astype(np.float32), R.astype(np.float32)


def kernel(A, WW, train_init, alpha, fract, lambd, l, A_y_list):
    from concourse.bass_utils import run_bass_kernel_spmd

    A = np.asarray(A, dtype=np.float32)
    WW = np.asarray(WW, dtype=np.float32)
    train_init = np.asarray(train_init, dtype=np.float32)

    P, Q, R = _host_coefs(
        np.asarray(alpha, np.float32), np.asarray(fract, np.float32),
        np.asarray(lambd, np.float32), np.asarray(l, np.float32))

    Wneg = -WW[:, :, 0]                         # [1200, 1200]

    xts, wcs, coefs = {}, {}, {}
    for beta in range(PB):
        bsl = slice(beta * BL, (beta + 1) * BL)
        xa = A[:, bsl, :, 0].transpose(2, 0, 1).reshape(N, CA)      # col=la*BL+b
        xu = train_init[bsl, :, :, 1].transpose(1, 2, 0).reshape(N, CU)  # col=t*BL+b
        XT = np.concatenate([xa, xu], axis=1)                       # [1200, 400]
        # partition-major: [KT, NK*C], col = k*C + c
        xts[beta] = np.ascontiguousarray(
            XT.reshape(NK, KT, C).transpose(1, 0, 2).reshape(KT, NK * C),
            dtype=np.float32)
    for g in range(PJ):
        gsl = slice(g * JL, (g + 1) * JL)
        # partition-major, jt-major: [KT, NJ*NK*JS], col = jt*NK*JS + k*JS + s
        wcs[g] = np.ascontiguousarray(
            Wneg[:, gsl].reshape(NK, KT, NJ, JS).transpose(1, 2, 0, 3)
            .reshape(KT, NK * JL), dtype=np.float32)
        # coef [JS, 108]: col = kind*36 + jt*12 + la
        kinds = [M[:, gsl].reshape(NL, NJ, JS).transpose(2, 1, 0)
                 for M in (P, Q, R)]                                # [100, 3, 12]
        coefs[g] = np.ascontiguousarray(
            np.stack(kinds, axis=1).reshape(JS, 3 * NJ * NL), dtype=np.float32)

    in_maps = []
    for core in range(PB * PJ):
        beta, g = divmod(core, PJ)
        in_maps.append({"xt": xts[beta], "wc": wcs[g], "coef": coefs[g]})

    nc = _get_nc()
    res = run_bass_kernel_spmd(nc, in_maps, core_ids=list(range(PB * PJ)))
    kernel.last_results = res

    full = np.empty((B, NL, N), dtype=np.float32)
    for core in range(PB * PJ):
        beta, g = divmod(core, PJ)
        o = res.results[core]["out"]            # [300, 192], col = la*BL+b
        full[beta * BL:(beta + 1) * BL, :, g * JL:(g + 1) * JL] = (
            o.reshape(JL, NL, BL).transpose(2, 1, 0))
    return full.reshape(B, NL, N, 1)


# revision 18
# speedup vs baseline: 1.0589x; 1.0589x over previous
# Trainium2 Bass kernel for nn_FCM_series_1 (gnn_message_passing).
#
# Math (derived from the reference):
#   aggregate(X, WW)[l,b,j] = tanh(-sum_i X[l,b,i] * WW[i,j])
#   T_A  = aggregate(A, WW)                     (12 lags x B rows)
#   U[t] = aggregate(train_init[:,:,t,1], WW)   (13 unique rows per batch;
#          A_N_OLD[la] = U[la], A_0_NEW[la] = U[la+1])
#   out[b,la,j] = P[la,j]*T_A[la,b,j] + Q[la,j]*U[la+1,b,j] + R[la,j]*U[la,b,j]
# with host-computable coefficients
#   P[la,j] = 2 * lambd[la, j%200] / belta[la] * 3**fract[la]
#   Q[la,j] = 3 * lambd[la, j%200] * l[la, j%200] / belta[la]
#   R[la,j] = Q[la,j] * Gamma(a+1)/(6*Gamma(a-2))
#   belta[la] = sum_{k=0..3} Gamma(a+1)/(Gamma(k+1)*Gamma(a-k+1))
#
# Sharding over 8 cores: batch split x2 (16 each), output node dim j split x4
# (300 each). Per core one matmul chain: lhsT=W-chunk tiles, rhs=X^T tiles,
# PSUM-accumulated over 10 k-tiles of 120, in float32r (single-pass fp32 PE
# mode, 4x faster than fp32 LOW_HIGH). W is negated on the host so psum
# already holds -X@W; tanh on ScalarE; coefficient combine on VectorE with
# 0-stride broadcast APs; per-core [300,192] result re-assembled on the host.
#
# HBM layouts are host-repacked to partition-major so every DMA descriptor is
# one large contiguous run per partition; input DMAs are split between the two
# HWDGE queues (sync for W, scalar for X) to double aggregate DMA throughput.

import math

import numpy as np

LAG = 13
B = 32
N = 1200
H = 1.0 / 3.0

PB = 2          # batch shards
PJ = 4          # j shards
BL = B // PB    # 16 batches per core
JL = N // PJ    # 300 output nodes per core
NL = LAG - 1    # 12
CA = NL * BL    # 192 cols: T_A block, col = la*BL + b
CU = LAG * BL   # 208 cols: U block,  col = CA + t*BL + b
C = CA + CU     # 400 matmul moving cols
KT = 120        # contraction tile
NK = N // KT    # 10
JS = 100        # j subtile (psum partition dim)
NJ = JL // JS   # 3
NCH = 2         # input DMA chunks per tensor (5 k-tiles each)

_cached = None


def _gamma(x):
    return math.gamma(x)


def _build_nc():
    import concourse.bacc as bacc
    import concourse.mybir as mybir
    from concourse.tile import TileContext

    f32 = mybir.dt.float32
    f32r = mybir.dt.float32r
    nc = bacc.Bacc(None, target_bir_lowering=False)

    # partition-major repacked inputs (see kernel() for layouts)
    xt = nc.dram_tensor("xt", [KT, NK * C], f32r, kind="ExternalInput")
    wc = nc.dram_tensor("wc", [KT, NK * JL], f32r, kind="ExternalInput")
    coef = nc.dram_tensor("coef", [JS, 3 * NJ * NL], f32, kind="ExternalInput")
    out = nc.dram_tensor("out", [JL, CA], f32, kind="ExternalOutput")

    with TileContext(nc) as tc:
        with (
            tc.tile_pool(name="sb", bufs=1) as pool,
            tc.tile_pool(name="ps", bufs=1, space="PSUM") as pspool,
        ):
            # Input streaming, k-major on both queues with a fine-grained
            # tail: the k9 chunks land last and are small, so only 3 matmuls
            # plus the epilogue remain after the final DMA byte.
            # sync:   Wk0-2 | Wk3-5 | Xk6-7 | Xk8 | Wk9
            # scalar: Xk0-2 | Xk3-5 | Wk6-8 | Xk9
            w_tiles = [None] * NK   # per-k [KT, JL] views
            x_tiles = [None] * NK   # per-k [KT, C] views

            def load(eng, kind, k0, nk, gi):
                dram, width, tl = (wc, JL, w_tiles) if kind == "w" \
                    else (xt, C, x_tiles)
                g = pool.tile([KT, nk * width], f32r, tag=f"g{gi}",
                              name=f"g{gi}")
                eng.dma_start(
                    out=g[:], in_=dram[:, k0 * width:(k0 + nk) * width])
                for kk in range(nk):
                    tl[k0 + kk] = g[:, kk * width:(kk + 1) * width]

            load(nc.sync, "w", 0, 3, 0)
            load(nc.scalar, "x", 0, 3, 1)
            load(nc.sync, "w", 3, 3, 2)
            load(nc.scalar, "x", 3, 3, 3)
            load(nc.sync, "x", 6, 2, 4)
            load(nc.scalar, "w", 6, 3, 5)
            load(nc.sync, "x", 8, 1, 6)
            load(nc.scalar, "x", 9, 1, 7)
            load(nc.sync, "w", 9, 1, 8)
            coef_all = pool.tile([JS, 3 * NJ * NL], f32, tag="coef")
            nc.gpsimd.dma_start(out=coef_all[:], in_=coef[:, :])

            ps = [pspool.tile([JS, C], f32, tag=f"ps{jt}", name=f"ps{jt}")
                  for jt in range(NJ)]
            for k in range(NK):
                for jt in range(NJ):
                    nc.tensor.matmul(
                        ps[jt][:],
                        w_tiles[k][:, jt * JS:(jt + 1) * JS],
                        x_tiles[k],
                        start=(k == 0), stop=(k == NK - 1),
                    )

            # Replicate the [JS, 12] coefficient vectors to [JS, 192] during
            # the DMA phase (DVE idle) so the combine ops run on flat APs.
            crep = pool.tile([JS, 3 * NJ * CA], f32, tag="crep")
            for i in range(3):
                for jt in range(NJ):
                    src = coef_all[:, i * NJ * NL + jt * NL:
                                   i * NJ * NL + (jt + 1) * NL]
                    dst = crep[:, (i * NJ + jt) * CA:(i * NJ + jt + 1) * CA]
                    nc.vector.tensor_copy(
                        dst.rearrange("p (l b) -> p l b", b=BL),
                        src.broadcast_to([JS, NL, BL]))

            # Per-jt epilogue, pipelined: tanh on ACT, flat combine on DVE
            # (jt0, jt2) / GpSimd (jt1), per-jt output DMA.
            t_all = pool.tile([JS, NJ * C], f32, tag="t")
            res = pool.tile([JS, NJ * CA], f32, tag="res")
            tmp = pool.tile([JS, NJ * CA], f32, tag="tmp")
            out3 = out.rearrange("(j p) c -> p j c", p=JS)
            for jt in range(NJ):
                # W was negated on the host, so psum = -(X @ W) already.
                nc.scalar.activation(
                    out=t_all[:, jt * C:(jt + 1) * C], in_=ps[jt][:],
                    func=mybir.ActivationFunctionType.Tanh,
                )
                t0 = jt * C
                tA = t_all[:, t0:t0 + CA]
                tU1 = t_all[:, t0 + CA + BL:t0 + CA + CU]
                tU0 = t_all[:, t0 + CA:t0 + CA + CA]
                rs = res[:, jt * CA:(jt + 1) * CA]
                ts = tmp[:, jt * CA:(jt + 1) * CA]
                cof = [crep[:, (i * NJ + jt) * CA:(i * NJ + jt + 1) * CA]
                       for i in range(3)]
                ve = nc.vector if jt != 1 else nc.gpsimd
                ve.tensor_mul(rs, cof[0], tA)
                ve.tensor_mul(ts, cof[1], tU1)
                ve.tensor_add(rs, rs, ts)
                ve.tensor_mul(ts, cof[2], tU0)
                ve.tensor_add(rs, rs, ts)
                oeng = nc.sync if jt != 1 else nc.scalar
                oeng.dma_start(out=out3[:, jt, :], in_=rs)

    return nc


def _get_nc():
    global _cached
    if _cached is None:
        _cached = _build_nc()
        _cached.finalize()   # Bacc: runs reg alloc + codegen passes
    return _cached


def _host_coefs(alpha, fract, lambd, l):
    # All [12,...] fp32; compute in float64, cast at the end.
    a = alpha[:, 0].astype(np.float64)          # [12]
    f = fract[:, 0].astype(np.float64)          # [12]
    lam = lambd[:, 0, :, 0].astype(np.float64)  # [12, 200]
    ll = l[:, 0, :, 0].astype(np.float64)       # [12, 200]

    belta = np.zeros(NL)
    for la in range(NL):
        g_a1 = _gamma(a[la] + 1.0)
        belta[la] = sum(
            g_a1 / (_gamma(kk + 1.0) * _gamma(a[la] - kk + 1.0)) for kk in range(4)
        )
    cN = np.array([_gamma(a[la] + 1.0) / (6.0 * _gamma(a[la] - 2.0))
                   for la in range(NL)])

    # tile lambda/l from 200 -> 1200 (index n % 200)
    lam_t = np.tile(lam, (1, 6))                # [12, 1200]
    ll_t = np.tile(ll, (1, 6))                  # [12, 1200]

    inv_hf = (1.0 / H) ** f                     # 3**fract
    P = 2.0 * lam_t / belta[:, None] * inv_hf[:, None]
    Q = lam_t * ll_t / belta[:, None] / H
    R = Q * cN[:, None]
    return P.astype(np.float32), Q.astype(np.float32), R.astype(np.float32)


def kernel(A, WW, train_init, alpha, fract, lambd, l, A_y_list):
    from concourse.bass_utils import run_bass_kernel_spmd

    A = np.asarray(A, dtype=np.float32)
    WW = np.asarray(WW, dtype=np.float32)
    train_init = np.asarray(train_init, dtype=np.float32)

    P, Q, R = _host_coefs(
        np.asarray(alpha, np.float32), np.asarray(fract, np.float32),
        np.asarray(lambd, np.float32), np.asarray(l, np.float32))

    Wneg = -WW[:, :, 0]                         # [1200, 1200]

    xts, wcs, coefs = {}, {}, {}
    for beta in range(PB):
        bsl = slice(beta * BL, (beta + 1) * BL)
        xa = A[:, bsl, :, 0].transpose(2, 0, 1).reshape(N, CA)      # col=la*BL+b
        xu = train_init[bsl, :, :, 1].transpose(1, 2, 0).reshape(N, CU)  # col=t*BL+b
        XT = np.concatenate([xa, xu], axis=1)                       # [1200, 400]
        # partition-major: [KT, NK*C], col = k*C + c
        xts[beta] = np.ascontiguousarray(
            XT.reshape(NK, KT, C).transpose(1, 0, 2).reshape(KT, NK * C),
            dtype=np.float32)
    for g in range(PJ):
        gsl = slice(g * JL, (g + 1) * JL)
        # partition-major: [KT, NK*JL], col = k*JL + j
        wcs[g] = np.ascontiguousarray(
            Wneg[:, gsl].reshape(NK, KT, JL).transpose(1, 0, 2)
            .reshape(KT, NK * JL), dtype=np.float32)
        # coef [JS, 108]: col = kind*36 + jt*12 + la
        kinds = [M[:, gsl].reshape(NL, NJ, JS).transpose(2, 1, 0)
                 for M in (P, Q, R)]                                # [100, 3, 12]
        coefs[g] = np.ascontiguousarray(
            np.stack(kinds, axis=1).reshape(JS, 3 * NJ * NL), dtype=np.float32)

    in_maps = []
    for core in range(PB * PJ):
        beta, g = divmod(core, PJ)
        in_maps.append({"xt": xts[beta], "wc": wcs[g], "coef": coefs[g]})

    nc = _get_nc()
    res = run_bass_kernel_spmd(nc, in_maps, core_ids=list(range(PB * PJ)))
    kernel.last_results = res

    full = np.empty((B, NL, N), dtype=np.float32)
    for core in range(PB * PJ):
        beta, g = divmod(core, PJ)
        o = res.results[core]["out"]            # [300, 192], col = la*BL+b
        full[beta * BL:(beta + 1) * BL, :, g * JL:(g + 1) * JL] = (
            o.reshape(JL, NL, BL).transpose(2, 1, 0))
    return full.reshape(B, NL, N, 1)


# revision 20
# speedup vs baseline: 1.0809x; 1.0208x over previous
# Trainium2 Bass kernel for nn_FCM_series_1 (gnn_message_passing).
#
# Math (derived from the reference):
#   aggregate(X, WW)[l,b,j] = tanh(-sum_i X[l,b,i] * WW[i,j])
#   T_A  = aggregate(A, WW)                     (12 lags x B rows)
#   U[t] = aggregate(train_init[:,:,t,1], WW)   (13 unique rows per batch;
#          A_N_OLD[la] = U[la], A_0_NEW[la] = U[la+1])
#   out[b,la,j] = P[la,j]*T_A[la,b,j] + Q[la,j]*U[la+1,b,j] + R[la,j]*U[la,b,j]
# with host-computable coefficients
#   P[la,j] = 2 * lambd[la, j%200] / belta[la] * 3**fract[la]
#   Q[la,j] = 3 * lambd[la, j%200] * l[la, j%200] / belta[la]
#   R[la,j] = Q[la,j] * Gamma(a+1)/(6*Gamma(a-2))
#   belta[la] = sum_{k=0..3} Gamma(a+1)/(Gamma(k+1)*Gamma(a-k+1))
#
# Sharding over 8 cores: batch split x2 (16 each), output node dim j split x4
# (300 each). Per core one matmul chain: lhsT=W-chunk tiles, rhs=X^T tiles,
# PSUM-accumulated over 10 k-tiles of 120, in float32r (single-pass fp32 PE
# mode, 4x faster than fp32 LOW_HIGH). W is negated on the host so psum
# already holds -X@W; tanh on ScalarE; coefficient combine on VectorE with
# 0-stride broadcast APs; per-core [300,192] result re-assembled on the host.
#
# HBM layouts are host-repacked to partition-major so every DMA descriptor is
# one large contiguous run per partition; input DMAs are split between the two
# HWDGE queues (sync for W, scalar for X) to double aggregate DMA throughput.

import math

import numpy as np

LAG = 13
B = 32
N = 1200
H = 1.0 / 3.0

PB = 2          # batch shards
PJ = 4          # j shards
BL = B // PB    # 16 batches per core
JL = N // PJ    # 300 output nodes per core
NL = LAG - 1    # 12
CA = NL * BL    # 192 cols: T_A block, col = la*BL + b
CU = LAG * BL   # 208 cols: U block,  col = CA + t*BL + b
C = CA + CU     # 400 matmul moving cols
KT = 120        # contraction tile
NK = N // KT    # 10
JS = 100        # j subtile (psum partition dim)
NJ = JL // JS   # 3
NCH = 2         # input DMA chunks per tensor (5 k-tiles each)

_cached = None


def _gamma(x):
    return math.gamma(x)


def _build_nc():
    import concourse.bacc as bacc
    import concourse.mybir as mybir
    from concourse.tile import TileContext

    f32 = mybir.dt.float32
    f32r = mybir.dt.float32r
    nc = bacc.Bacc(None, target_bir_lowering=False)

    # partition-major repacked inputs (see kernel() for layouts)
    xt = nc.dram_tensor("xt", [KT, NK * C], f32r, kind="ExternalInput")
    wc = nc.dram_tensor("wc", [KT, NK * JL], f32r, kind="ExternalInput")
    coef = nc.dram_tensor("coef", [JS, 3 * NJ * NL], f32, kind="ExternalInput")
    out = nc.dram_tensor("out", [JL, CA], f32, kind="ExternalOutput")

    with TileContext(nc) as tc:
        with (
            tc.tile_pool(name="sb", bufs=1) as pool,
            tc.tile_pool(name="ps", bufs=1, space="PSUM") as pspool,
        ):
            # Input streaming, k-major on both queues with a fine-grained
            # tail: the k9 chunks land last and are small, so only 3 matmuls
            # plus the epilogue remain after the final DMA byte.
            # sync:   Wk0-2 | Wk3-5 | Xk6-7 | Xk8 | Wk9
            # scalar: Xk0-2 | Xk3-5 | Wk6-8 | Xk9
            w_tiles = [None] * NK   # per-k [KT, JL] views
            x_tiles = [None] * NK   # per-k [KT, C] views

            def load(eng, kind, k0, nk, gi):
                dram, width, tl = (wc, JL, w_tiles) if kind == "w" \
                    else (xt, C, x_tiles)
                g = pool.tile([KT, nk * width], f32r, tag=f"g{gi}",
                              name=f"g{gi}")
                eng.dma_start(
                    out=g[:], in_=dram[:, k0 * width:(k0 + nk) * width])
                for kk in range(nk):
                    tl[k0 + kk] = g[:, kk * width:(kk + 1) * width]

            load(nc.sync, "w", 0, 3, 0)
            load(nc.scalar, "x", 0, 3, 1)
            load(nc.sync, "w", 3, 3, 2)
            load(nc.scalar, "x", 3, 3, 3)
            load(nc.sync, "x", 6, 2, 4)
            load(nc.scalar, "w", 6, 3, 5)
            load(nc.sync, "x", 8, 1, 6)
            load(nc.scalar, "x", 9, 1, 7)
            load(nc.sync, "w", 9, 1, 8)
            coef_all = pool.tile([JS, 3 * NJ * NL], f32, tag="coef")
            nc.gpsimd.dma_start(out=coef_all[:], in_=coef[:, :])

            ps = [pspool.tile([JS, C], f32, tag=f"ps{jt}", name=f"ps{jt}")
                  for jt in range(NJ)]
            for k in range(NK):
                for jt in range(NJ):
                    nc.tensor.matmul(
                        ps[jt][:],
                        w_tiles[k][:, jt * JS:(jt + 1) * JS],
                        x_tiles[k],
                        start=(k == 0), stop=(k == NK - 1),
                    )

            # Replicate the [JS, 12] coefficient vectors to [JS, 192] during
            # the DMA phase (DVE idle) so the combine ops run on flat APs.
            crep = pool.tile([JS, 3 * NJ * CA], f32, tag="crep")
            for i in range(3):
                for jt in range(NJ):
                    src = coef_all[:, i * NJ * NL + jt * NL:
                                   i * NJ * NL + (jt + 1) * NL]
                    dst = crep[:, (i * NJ + jt) * CA:(i * NJ + jt + 1) * CA]
                    nc.gpsimd.tensor_copy(
                        dst.rearrange("p (l b) -> p l b", b=BL),
                        src.broadcast_to([JS, NL, BL]))

            # Per-jt epilogue, pipelined: tanh on ACT, flat combine on DVE
            # (jt0, jt2) / GpSimd (jt1), per-jt output DMA.
            t_all = pool.tile([JS, NJ * C], f32, tag="t")
            res = pool.tile([JS, NJ * CA], f32, tag="res")
            tmp = pool.tile([JS, NJ * CA], f32, tag="tmp")
            tmp2 = pool.tile([JS, NJ * CA], f32, tag="tmp2")
            out3 = out.rearrange("(j p) c -> p j c", p=JS)
            for jt in range(NJ):
                # W was negated on the host, so psum = -(X @ W) already.
                nc.scalar.activation(
                    out=t_all[:, jt * C:(jt + 1) * C], in_=ps[jt][:],
                    func=mybir.ActivationFunctionType.Tanh,
                )
                t0 = jt * C
                tA = t_all[:, t0:t0 + CA]
                tU1 = t_all[:, t0 + CA + BL:t0 + CA + CU]
                tU0 = t_all[:, t0 + CA:t0 + CA + CA]
                rs = res[:, jt * CA:(jt + 1) * CA]
                ts = tmp[:, jt * CA:(jt + 1) * CA]
                ts2 = tmp2[:, jt * CA:(jt + 1) * CA]
                cof = [crep[:, (i * NJ + jt) * CA:(i * NJ + jt + 1) * CA]
                       for i in range(3)]
                ve = nc.vector if jt != 1 else nc.gpsimd
                # three independent muls (pipeline on the engine), then adds
                ve.tensor_mul(rs, cof[0], tA)
                ve.tensor_mul(ts, cof[1], tU1)
                ve.tensor_mul(ts2, cof[2], tU0)
                ve.tensor_add(rs, rs, ts)
                ve.tensor_add(rs, rs, ts2)
                oeng = nc.sync if jt != 1 else nc.scalar
                oeng.dma_start(out=out3[:, jt, :], in_=rs)

    return nc


def _get_nc():
    global _cached
    if _cached is None:
        _cached = _build_nc()
        _cached.finalize()   # Bacc: runs reg alloc + codegen passes
    return _cached


def _host_coefs(alpha, fract, lambd, l):
    # All [12,...] fp32; compute in float64, cast at the end.
    a = alpha[:, 0].astype(np.float64)          # [12]
    f = fract[:, 0].astype(np.float64)          # [12]
    lam = lambd[:, 0, :, 0].astype(np.float64)  # [12, 200]
    ll = l[:, 0, :, 0].astype(np.float64)       # [12, 200]

    belta = np.zeros(NL)
    for la in range(NL):
        g_a1 = _gamma(a[la] + 1.0)
        belta[la] = sum(
            g_a1 / (_gamma(kk + 1.0) * _gamma(a[la] - kk + 1.0)) for kk in range(4)
        )
    cN = np.array([_gamma(a[la] + 1.0) / (6.0 * _gamma(a[la] - 2.0))
                   for la in range(NL)])

    # tile lambda/l from 200 -> 1200 (index n % 200)
    lam_t = np.tile(lam, (1, 6))                # [12, 1200]
    ll_t = np.tile(ll, (1, 6))                  # [12, 1200]

    inv_hf = (1.0 / H) ** f                     # 3**fract
    P = 2.0 * lam_t / belta[:, None] * inv_hf[:, None]
    Q = lam_t * ll_t / belta[:, None] / H
    R = Q * cN[:, None]
    return P.astype(np.float32), Q.astype(np.float32), R.astype(np.float32)


def kernel(A, WW, train_init, alpha, fract, lambd, l, A_y_list):
    from concourse.bass_utils import run_bass_kernel_spmd

    A = np.asarray(A, dtype=np.float32)
    WW = np.asarray(WW, dtype=np.float32)
    train_init = np.asarray(train_init, dtype=np.float32)

    P, Q, R = _host_coefs(
        np.asarray(alpha, np.float32), np.asarray(fract, np.float32),
        np.asarray(lambd, np.float32), np.asarray(l, np.float32))

    Wneg = -WW[:, :, 0]                         # [1200, 1200]

    xts, wcs, coefs = {}, {}, {}
    for beta in range(PB):
        bsl = slice(beta * BL, (beta + 1) * BL)
        xa = A[:, bsl, :, 0].transpose(2, 0, 1).reshape(N, CA)      # col=la*BL+b
        xu = train_init[bsl, :, :, 1].transpose(1, 2, 0).reshape(N, CU)  # col=t*BL+b
        XT = np.concatenate([xa, xu], axis=1)                       # [1200, 400]
        # partition-major: [KT, NK*C], col = k*C + c
        xts[beta] = np.ascontiguousarray(
            XT.reshape(NK, KT, C).transpose(1, 0, 2).reshape(KT, NK * C),
            dtype=np.float32)
    for g in range(PJ):
        gsl = slice(g * JL, (g + 1) * JL)
        # partition-major: [KT, NK*JL], col = k*JL + j
        wcs[g] = np.ascontiguousarray(
            Wneg[:, gsl].reshape(NK, KT, JL).transpose(1, 0, 2)
            .reshape(KT, NK * JL), dtype=np.float32)
        # coef [JS, 108]: col = kind*36 + jt*12 + la
        kinds = [M[:, gsl].reshape(NL, NJ, JS).transpose(2, 1, 0)
                 for M in (P, Q, R)]                                # [100, 3, 12]
        coefs[g] = np.ascontiguousarray(
            np.stack(kinds, axis=1).reshape(JS, 3 * NJ * NL), dtype=np.float32)

    in_maps = []
    for core in range(PB * PJ):
        beta, g = divmod(core, PJ)
        in_maps.append({"xt": xts[beta], "wc": wcs[g], "coef": coefs[g]})

    nc = _get_nc()
    res = run_bass_kernel_spmd(nc, in_maps, core_ids=list(range(PB * PJ)))
    kernel.last_results = res

    full = np.empty((B, NL, N), dtype=np.float32)
    for core in range(PB * PJ):
        beta, g = divmod(core, PJ)
        o = res.results[core]["out"]            # [300, 192], col = la*BL+b
        full[beta * BL:(beta + 1) * BL, :, g * JL:(g + 1) * JL] = (
            o.reshape(JL, NL, BL).transpose(2, 1, 0))
    return full.reshape(B, NL, N, 1)


# revision 26
# speedup vs baseline: 1.3417x; 1.2413x over previous
# Trainium2 Bass kernel for nn_FCM_series_1 (gnn_message_passing).
#
# Math (derived from the reference):
#   aggregate(X, WW)[l,b,j] = tanh(-sum_i X[l,b,i] * WW[i,j])
#   T_A  = aggregate(A, WW)                     (12 lags x B rows)
#   U[t] = aggregate(train_init[:,:,t,1], WW)   (13 unique rows per batch;
#          A_N_OLD[la] = U[la], A_0_NEW[la] = U[la+1])
#   out[b,la,j] = P[la,j]*T_A[la,b,j] + Q[la,j]*U[la+1,b,j] + R[la,j]*U[la,b,j]
# with host-computable coefficients
#   P[la,j] = 2 * lambd[la, j%200] / belta[la] * 3**fract[la]
#   Q[la,j] = 3 * lambd[la, j%200] * l[la, j%200] / belta[la]
#   R[la,j] = Q[la,j] * Gamma(a+1)/(6*Gamma(a-2))
#   belta[la] = sum_{k=0..3} Gamma(a+1)/(Gamma(k+1)*Gamma(a-k+1))
#
# Sharding over 8 cores: batch split x2 (16 each), output node dim j split x4
# (300 each). Per core one matmul chain: lhsT=W-chunk tiles, rhs=X^T tiles,
# PSUM-accumulated over 10 k-tiles of 120, in float32r (single-pass fp32 PE
# mode, 4x faster than fp32 LOW_HIGH). W is negated on the host so psum
# already holds -X@W; tanh on ScalarE; coefficient combine on VectorE with
# 0-stride broadcast APs; per-core [300,192] result re-assembled on the host.
#
# HBM layouts are host-repacked to partition-major so every DMA descriptor is
# one large contiguous run per partition; input DMAs are split between the two
# HWDGE queues (sync for W, scalar for X) to double aggregate DMA throughput.

import math

import numpy as np

LAG = 13
B = 32
N = 1200
H = 1.0 / 3.0

PB = 2          # batch shards
PJ = 4          # j shards
BL = B // PB    # 16 batches per core
JL = N // PJ    # 300 output nodes per core
NL = LAG - 1    # 12
CA = NL * BL    # 192 cols: T_A block, col = la*BL + b
CU = LAG * BL   # 208 cols: U block,  col = CA + t*BL + b
C = CA + CU     # 400 matmul moving cols
KT = 120        # contraction tile
NK = N // KT    # 10
JS = 100        # j subtile (psum partition dim)
NJ = JL // JS   # 3
NCH = 2         # input DMA chunks per tensor (5 k-tiles each)
USE_BF16 = True  # matmul operand dtype: bf16 halves DMA bytes, ~2.5e-3 rel err

_cached = None


def _gamma(x):
    return math.gamma(x)


def _build_nc():
    import concourse.bacc as bacc
    import concourse.mybir as mybir
    from concourse.tile import TileContext

    f32 = mybir.dt.float32
    f32r = mybir.dt.bfloat16 if USE_BF16 else mybir.dt.float32r
    nc = bacc.Bacc(None, target_bir_lowering=False)

    # partition-major repacked inputs (see kernel() for layouts)
    xt = nc.dram_tensor("xt", [KT, NK * C], f32r, kind="ExternalInput")
    wc = nc.dram_tensor("wc", [KT, NK * JL], f32r, kind="ExternalInput")
    coef = nc.dram_tensor("coef", [JS, 3 * NJ * NL], f32, kind="ExternalInput")
    out = nc.dram_tensor("out", [JL, CA], f32, kind="ExternalOutput")

    with TileContext(nc) as tc:
        with (
            tc.tile_pool(name="sb", bufs=1) as pool,
            tc.tile_pool(name="ps", bufs=1, space="PSUM") as pspool,
        ):
            # Input streaming, k-major on both queues with a fine-grained
            # tail: the k9 chunks land last and are small, so only 3 matmuls
            # plus the epilogue remain after the final DMA byte.
            # sync:   Wk0-2 | Wk3-5 | Xk6-7 | Xk8 | Wk9
            # scalar: Xk0-2 | Xk3-5 | Wk6-8 | Xk9
            w_tiles = [None] * NK   # per-k [KT, JL] views
            x_tiles = [None] * NK   # per-k [KT, C] views

            def load(eng, kind, k0, nk, gi):
                dram, width, tl = (wc, JL, w_tiles) if kind == "w" \
                    else (xt, C, x_tiles)
                g = pool.tile([KT, nk * width], f32r, tag=f"g{gi}",
                              name=f"g{gi}")
                eng.dma_start(
                    out=g[:], in_=dram[:, k0 * width:(k0 + nk) * width])
                for kk in range(nk):
                    tl[k0 + kk] = g[:, kk * width:(kk + 1) * width]

            load(nc.sync, "w", 0, 3, 0)
            load(nc.scalar, "x", 0, 3, 1)
            load(nc.sync, "w", 3, 3, 2)
            load(nc.scalar, "x", 3, 3, 3)
            load(nc.sync, "x", 6, 2, 4)
            load(nc.scalar, "w", 6, 3, 5)
            load(nc.sync, "x", 8, 1, 6)
            load(nc.scalar, "x", 9, 1, 7)
            load(nc.sync, "w", 9, 1, 8)
            coef_all = pool.tile([JS, 3 * NJ * NL], f32, tag="coef")
            nc.gpsimd.dma_start(out=coef_all[:], in_=coef[:, :])

            ps = [pspool.tile([JS, C], f32, tag=f"ps{jt}", name=f"ps{jt}")
                  for jt in range(NJ)]
            for k in range(NK):
                for jt in range(NJ):
                    nc.tensor.matmul(
                        ps[jt][:],
                        w_tiles[k][:, jt * JS:(jt + 1) * JS],
                        x_tiles[k],
                        start=(k == 0), stop=(k == NK - 1),
                    )

            # Replicate the [JS, 12] coefficient vectors to [JS, 192] during
            # the DMA phase (DVE idle) so the combine ops run on flat APs.
            crep = pool.tile([JS, 3 * NJ * CA], f32, tag="crep")
            for i in range(3):
                for jt in range(NJ):
                    src = coef_all[:, i * NJ * NL + jt * NL:
                                   i * NJ * NL + (jt + 1) * NL]
                    dst = crep[:, (i * NJ + jt) * CA:(i * NJ + jt + 1) * CA]
                    nc.gpsimd.tensor_copy(
                        dst.rearrange("p (l b) -> p l b", b=BL),
                        src.broadcast_to([JS, NL, BL]))

            # Per-jt epilogue, pipelined: tanh on ACT, flat combine on DVE
            # (jt0, jt2) / GpSimd (jt1), per-jt output DMA.
            t_all = pool.tile([JS, NJ * C], f32, tag="t")
            res = pool.tile([JS, NJ * CA], f32, tag="res")
            tmp = pool.tile([JS, NJ * CA], f32, tag="tmp")
            tmp2 = pool.tile([JS, NJ * CA], f32, tag="tmp2")
            out3 = out.rearrange("(j p) c -> p j c", p=JS)
            for jt in range(NJ):
                # W was negated on the host, so psum = -(X @ W) already.
                nc.scalar.activation(
                    out=t_all[:, jt * C:(jt + 1) * C], in_=ps[jt][:],
                    func=mybir.ActivationFunctionType.Tanh,
                )
                t0 = jt * C
                tA = t_all[:, t0:t0 + CA]
                tU1 = t_all[:, t0 + CA + BL:t0 + CA + CU]
                tU0 = t_all[:, t0 + CA:t0 + CA + CA]
                rs = res[:, jt * CA:(jt + 1) * CA]
                ts = tmp[:, jt * CA:(jt + 1) * CA]
                ts2 = tmp2[:, jt * CA:(jt + 1) * CA]
                cof = [crep[:, (i * NJ + jt) * CA:(i * NJ + jt + 1) * CA]
                       for i in range(3)]
                ve = nc.vector if jt != 1 else nc.gpsimd
                # three independent muls (pipeline on the engine), then adds
                ve.tensor_mul(rs, cof[0], tA)
                ve.tensor_mul(ts, cof[1], tU1)
                ve.tensor_mul(ts2, cof[2], tU0)
                ve.tensor_add(rs, rs, ts)
                ve.tensor_add(rs, rs, ts2)
                oeng = nc.sync if jt != 1 else nc.scalar
                oeng.dma_start(out=out3[:, jt, :], in_=rs)

    return nc


def _get_nc():
    global _cached
    if _cached is None:
        _cached = _build_nc()
        _cached.finalize()   # Bacc: runs reg alloc + codegen passes
    return _cached


def _host_coefs(alpha, fract, lambd, l):
    # All [12,...] fp32; compute in float64, cast at the end.
    a = alpha[:, 0].astype(np.float64)          # [12]
    f = fract[:, 0].astype(np.float64)          # [12]
    lam = lambd[:, 0, :, 0].astype(np.float64)  # [12, 200]
    ll = l[:, 0, :, 0].astype(np.float64)       # [12, 200]

    belta = np.zeros(NL)
    for la in range(NL):
        g_a1 = _gamma(a[la] + 1.0)
        belta[la] = sum(
            g_a1 / (_gamma(kk + 1.0) * _gamma(a[la] - kk + 1.0)) for kk in range(4)
        )
    cN = np.array([_gamma(a[la] + 1.0) / (6.0 * _gamma(a[la] - 2.0))
                   for la in range(NL)])

    # tile lambda/l from 200 -> 1200 (index n % 200)
    lam_t = np.tile(lam, (1, 6))                # [12, 1200]
    ll_t = np.tile(ll, (1, 6))                  # [12, 1200]

    inv_hf = (1.0 / H) ** f                     # 3**fract
    P = 2.0 * lam_t / belta[:, None] * inv_hf[:, None]
    Q = lam_t * ll_t / belta[:, None] / H
    R = Q * cN[:, None]
    return P.astype(np.float32), Q.astype(np.float32), R.astype(np.float32)


def kernel(A, WW, train_init, alpha, fract, lambd, l, A_y_list):
    from concourse.bass_utils import run_bass_kernel_spmd

    if USE_BF16:
        import ml_dtypes
        mm_dt = ml_dtypes.bfloat16
    else:
        mm_dt = np.float32

    A = np.asarray(A, dtype=np.float32)
    WW = np.asarray(WW, dtype=np.float32)
    train_init = np.asarray(train_init, dtype=np.float32)

    P, Q, R = _host_coefs(
        np.asarray(alpha, np.float32), np.asarray(fract, np.float32),
        np.asarray(lambd, np.float32), np.asarray(l, np.float32))

    Wneg = -WW[:, :, 0]                         # [1200, 1200]

    xts, wcs, coefs = {}, {}, {}
    for beta in range(PB):
        bsl = slice(beta * BL, (beta + 1) * BL)
        xa = A[:, bsl, :, 0].transpose(2, 0, 1).reshape(N, CA)      # col=la*BL+b
        xu = train_init[bsl, :, :, 1].transpose(1, 2, 0).reshape(N, CU)  # col=t*BL+b
        XT = np.concatenate([xa, xu], axis=1)                       # [1200, 400]
        # partition-major: [KT, NK*C], col = k*C + c
        xts[beta] = np.ascontiguousarray(
            XT.reshape(NK, KT, C).transpose(1, 0, 2).reshape(KT, NK * C),
            dtype=mm_dt)
    for g in range(PJ):
        gsl = slice(g * JL, (g + 1) * JL)
        # partition-major: [KT, NK*JL], col = k*JL + j
        wcs[g] = np.ascontiguousarray(
            Wneg[:, gsl].reshape(NK, KT, JL).transpose(1, 0, 2)
            .reshape(KT, NK * JL), dtype=mm_dt)
        # coef [JS, 108]: col = kind*36 + jt*12 + la
        kinds = [M[:, gsl].reshape(NL, NJ, JS).transpose(2, 1, 0)
                 for M in (P, Q, R)]                                # [100, 3, 12]
        coefs[g] = np.ascontiguousarray(
            np.stack(kinds, axis=1).reshape(JS, 3 * NJ * NL), dtype=np.float32)

    in_maps = []
    for core in range(PB * PJ):
        beta, g = divmod(core, PJ)
        in_maps.append({"xt": xts[beta], "wc": wcs[g], "coef": coefs[g]})

    nc = _get_nc()
    res = run_bass_kernel_spmd(nc, in_maps, core_ids=list(range(PB * PJ)))
    kernel.last_results = res

    full = np.empty((B, NL, N), dtype=np.float32)
    for core in range(PB * PJ):
        beta, g = divmod(core, PJ)
        o = res.results[core]["out"]            # [300, 192], col = la*BL+b
        full[beta * BL:(beta + 1) * BL, :, g * JL:(g + 1) * JL] = (
            o.reshape(JL, NL, BL).transpose(2, 1, 0))
    return full.reshape(B, NL, N, 1)
